# revision 1
# baseline (speedup 1.0000x reference)
"""KAN B-spline activation kernel for Trainium2 (8 NeuronCores, data-parallel on batch).

Math (validated vs reference to ~1e-7 rel):
  grid is uniform: g[t] = -1 + (t-3)*h, h = 0.125, t = 0..22; u = (x - g[0])/h = 8x + 11.
  For x in [0,1) only knot-window t in [8,18] has nonzero cubic bases.
  Let A[k] = x - g[8+k], k = 0..15 (k=15 unused pad).
  B1[m]  = Relu(1 - |A[m+1]|/h)                     (hat; == Cox-de Boor level 1), m=0..12
  B2d[m] = A[m]*B1[m]   - A[m+3]*B1[m+1]           (= 2h * B2), m=0..11
  B3d[m] = A[m]*B2d[m]  - A[m+4]*B2d[m+1]          (= 6h^2 * B3), m=0..10
  out[b,o,i] = sum_m B3d[b,i,m] * coef[o,i,8+m] / (6 h^2)

Device layout (per core, 128 batch rows in partitions):
  A/B* tiles: (128, 64 inputs x 16-knot-window blocks) in the free dim.
  B3 (128, 64*16) -> 8 PE transposes of 128-col groups (8 inputs each) ->
  basesT (K=(input,knot) partitions, batch free). Matmul per (group g, subgroup s):
  K=32 (2 inputs x 16 knots), N=128 (2 inputs x 64 outs), block-diagonal rhs built
  on host with the zeros/padding baked in. PSUM bank per group (128,512) is copied
  verbatim to SBUF and DMA'd out; host un-permutes (b, g, s, p, o) -> (b, o, i).
"""

import numpy as np
from contextlib import ExitStack

import concourse.bass as bass
import concourse.tile as tile
from concourse import bacc, mybir
from concourse.bass_utils import run_bass_kernel_spmd
from concourse.masks import make_identity

N_CORES = 8
B_TOT, IN_DIM, OUT_DIM = 1024, 64, 64
BPC = B_TOT // N_CORES          # 128 batch rows per core
K16 = 16                        # padded knot window per input
NG = 8                          # groups of 8 inputs
F32 = mybir.dt.float32

# If the stride-0 broadcast read on DVE fails, flip to False (log-doubling copies).
# HW faulted with stride-0 input APs on DVE (sim accepts them); use doubling.
USE_STRIDE0 = False

_CACHE = {}


def _build_nc():
    nc = bacc.Bacc("TRN2", target_bir_lowering=False, debug=False,
                   num_devices=N_CORES)
    x_d = nc.dram_tensor("x_in", [BPC, IN_DIM], F32, kind="ExternalInput").ap()
    rhs_d = nc.dram_tensor("rhs_in", [128, NG * 512], F32, kind="ExternalInput").ap()
    g3_d = nc.dram_tensor("g3_in", [1, IN_DIM * K16], F32, kind="ExternalInput").ap()
    out_d = nc.dram_tensor("out", [BPC, NG, 512], F32, kind="ExternalOutput").ap()

    with tile.TileContext(nc) as tc, ExitStack() as ctx:
        pool = ctx.enter_context(tc.tile_pool(name="main", bufs=1))
        psT = ctx.enter_context(tc.tile_pool(name="psT", bufs=2, space="PSUM"))
        psO = ctx.enter_context(tc.tile_pool(name="psO", bufs=4, space="PSUM"))
        og_pool = ctx.enter_context(tc.tile_pool(name="og", bufs=4))

        ident = pool.tile([128, 128], F32)
        make_identity(nc, ident)

        x_sb = pool.tile([BPC, IN_DIM], F32)
        nc.sync.dma_start(out=x_sb[:], in_=x_d)
        rhs_sb = pool.tile([128, NG * 512], F32)
        nc.sync.dma_start(out=rhs_sb[:], in_=rhs_d)
        # broadcast the (1, 1024) knot row across 128 partitions during DMA
        g3_sb = pool.tile([128, IN_DIM * K16], F32)
        g3_bcast = bass.AP(tensor=g3_d.tensor, offset=g3_d.offset,
                           ap=[[0, 128]] + list(g3_d.ap[1:]))
        nc.gpsimd.dma_start(out=g3_sb[:], in_=g3_bcast)
        g3v = g3_sb[:].rearrange("p (i k) -> p i k", k=K16)

        # broadcast x along the 16-knot window by log-doubling copies
        xt = pool.tile([BPC, IN_DIM, K16], F32)
        nc.vector.tensor_copy(xt[:, :, 0:1],
                              x_sb[:].rearrange("p (i k) -> p i k", k=1))
        w = 1
        while w < K16:
            n = min(w, K16 - w)
            nc.vector.tensor_copy(xt[:, :, w:w + n], xt[:, :, 0:n])
            w += n

        halves = ctx.enter_context(tc.tile_pool(name="halves", bufs=2))
        basesT = pool.tile([128, NG * 128], F32)
        HW_IN = IN_DIM // 2                       # 32 inputs per half
        for H in range(2):
            isl = slice(H * HW_IN, (H + 1) * HW_IN)
            Ah = halves.tile([BPC, HW_IN, K16], F32)
            nc.vector.tensor_sub(Ah[:], xt[:, isl, :], g3v[:, isl, :])
            Bab = halves.tile([BPC, HW_IN, 13], F32)
            nc.scalar.activation(out=Bab[:], in_=Ah[:, :, 1:14],
                                 func=mybir.ActivationFunctionType.Abs)
            B1h = halves.tile([BPC, HW_IN, 13], F32)
            # Relu(-8*|A| + 1) == Relu(1 - |A|/h)
            nc.scalar.activation(out=B1h[:], in_=Bab[:],
                                 func=mybir.ActivationFunctionType.Relu,
                                 scale=-8.0, bias=1.0)
            Ml2 = halves.tile([BPC, HW_IN, 12], F32)
            Mr2 = halves.tile([BPC, HW_IN, 12], F32)
            B2h = halves.tile([BPC, HW_IN, 12], F32)
            nc.vector.tensor_mul(Ml2[:], Ah[:, :, 0:12], B1h[:, :, 0:12])
            nc.vector.tensor_mul(Mr2[:], Ah[:, :, 3:15], B1h[:, :, 1:13])
            nc.vector.tensor_sub(B2h[:], Ml2[:], Mr2[:])
            Ml3 = halves.tile([BPC, HW_IN, 11], F32)
            Mr3 = halves.tile([BPC, HW_IN, 11], F32)
            B3h = halves.tile([BPC, HW_IN, K16], F32)
            nc.vector.tensor_mul(Ml3[:], Ah[:, :, 0:11], B2h[:, :, 0:11])
            nc.vector.tensor_mul(Mr3[:], Ah[:, :, 4:15], B2h[:, :, 1:12])
            # pad knots 11..15 must be 0: they feed the transpose, whose
            # output multiplies real coef columns.
            nc.vector.memset(B3h[:, :, 11:16], 0.0)
            nc.vector.tensor_sub(B3h[:, :, 0:11], Ml3[:], Mr3[:])

            B3f = B3h[:].rearrange("p i k -> p (i k)")
            ps_t = psT.tile([128, 512], F32)
            for q in range(4):
                nc.tensor.transpose(out=ps_t[:, q * 128:(q + 1) * 128],
                                    in_=B3f[:, q * 128:(q + 1) * 128],
                                    identity=ident[:])
            dst = basesT[:, H * 512:(H + 1) * 512]
            if H == 0:
                nc.vector.tensor_copy(dst, ps_t[:])
            else:
                nc.scalar.copy(dst, ps_t[:])

            for q in range(4):
                g = 4 * H + q
                ps_o = psO.tile([128, 512], F32)
                nc.tensor.matmul(out=ps_o[:],
                                 lhsT=basesT[:, g * 128:(g + 1) * 128],
                                 rhs=rhs_sb[:, g * 512:(g + 1) * 512],
                                 start=True, stop=True)
                og = og_pool.tile([128, 512], F32)
                if g % 2 == 0:
                    nc.vector.tensor_copy(og[:], ps_o[:])
                else:
                    nc.scalar.copy(og[:], ps_o[:])
                nc.sync.dma_start(out=out_d[:, g, :], in_=og[:])

    nc.compile()
    return nc


def _host_inputs(x, coef, grid):
    x = np.ascontiguousarray(np.asarray(x, dtype=np.float32))
    coef = np.asarray(coef, dtype=np.float32)
    knots = np.asarray(grid, dtype=np.float32)[0, 0, :]          # (23,)
    h = float(knots[1] - knots[0])

    g3 = np.empty(K16, dtype=np.float32)
    g3[:15] = knots[8:23]
    g3[15] = knots[22] + h                                       # unused pad
    g3row = np.tile(g3, IN_DIM)[None, :]                         # (1, 1024)

    scale = 1.0 / (6.0 * h * h)
    cf = coef[:, :, 8:19] * scale                                # (o, i, 11)
    # block-diagonal rhs per group: rows (i_l,j) x cols (i_l', o), K=128, N=512
    rhs = np.zeros((128, NG * 512), dtype=np.float32)
    for i_l in range(8):
        for g in range(NG):
            i = g * 8 + i_l
            rhs[i_l * 16:i_l * 16 + 11,
                g * 512 + i_l * 64:g * 512 + i_l * 64 + 64] = cf[:, i, :].T
    return x, rhs, g3row


def _execute(x, coef, grid, trace=False, **spmd_kwargs):
    xf, rhs, g3row = _host_inputs(x, coef, grid)
    if "nc" not in _CACHE:
        _CACHE["nc"] = _build_nc()
    nc = _CACHE["nc"]
    in_maps = [{"x_in": np.ascontiguousarray(xf[c * BPC:(c + 1) * BPC]),
                "rhs_in": rhs, "g3_in": g3row} for c in range(N_CORES)]
    res = run_bass_kernel_spmd(nc, in_maps, list(range(N_CORES)),
                               trace=trace, **spmd_kwargs)
    full = np.empty((B_TOT, OUT_DIM, IN_DIM), dtype=np.float32)
    for c in range(N_CORES):
        t = res.results[c]["out"].reshape(BPC, NG, 8, 64)        # (b, g, i_l, o)
        full[c * BPC:(c + 1) * BPC] = (
            t.transpose(0, 3, 1, 2).reshape(BPC, OUT_DIM, IN_DIM))
    return full, res


def kernel(x, coef, grid):
    out, _ = _execute(x, coef, grid, trace=False)
    return out



# revision 4
# speedup vs baseline: 1.4646x; 1.4646x over previous
"""KAN B-spline activation kernel for Trainium2 (8 NeuronCores, data-parallel on batch).

Truncated-power-basis formulation (validated vs reference to ~3.8e-5 rel):
  Uniform grid: knots[t] = (t-3)*h - 1, h = 0.125. For x in [0,1) the cubic
  spline sum over the 11-coef window equals
      out[b,o,i] = sum_{n=0..10} D[o,i,n] * Relu(u[b,i] - n)^3,
  where u = (x - knots[8])/h  (= 8x + 3) and
  D[o,i,n] = (1/6) * sum_j w_j * coef[o,i,8+n-j], w = [1,-4,6,-4,1]
  (the h^3 factors cancel exactly). D and u are built on host.

Device layout (per core, 128 batch rows):
  - xu2 [9, 1024]: row k<8 = u[g*8+k, b] laid out (g,b); row 8 = ones.
  - sel [9, 128] const: sel[k, p] = (p//16 == k); sel[8, p] = -(p%16).
  - One matmul (lhsT=sel, rhs=xu2 half) -> PSUM t[p=(i_l,n), (g,b)] = u - n,
    i.e. the bases arrive already transposed; no per-group PE transposes.
  - R^3 = Relu(t) * (t*t)  (scalar relu + 2 vector muls) -> basesT in SBUF.
  - 8 matmuls: lhsT = basesT[:, g*128:...] (K=(i_l,n)=128, M=b=128),
    rhs = block-diagonal D [128, 512] per group, PSUM -> SBUF -> DRAM.
  - Block-diag rhs built on device: memset zeros + 8 scattered DMAs from the
    compact 256KB Dcomp DRAM tensor (only nonzero bands stored).
  - A few tiny warmup matmuls keep the PE busy from t=0 so the power-state
    ramp is over before the real matmuls issue.
"""

import numpy as np
from contextlib import ExitStack

import concourse.bass as bass
import concourse.tile as tile
from concourse import bacc, mybir
from concourse.bass_utils import run_bass_kernel_spmd

N_CORES = 8
B_TOT, IN_DIM, OUT_DIM = 1024, 64, 64
BPC = B_TOT // N_CORES          # 128 batch rows per core
K16 = 16                        # padded knot window per input
NG = 8                          # groups of 8 inputs
F32 = mybir.dt.float32
NWARM = 14                      # PE warmup matmuls (power-state ramp)

_CACHE = {}


def _build_nc():
    nc = bacc.Bacc("TRN2", target_bir_lowering=False, debug=False,
                   num_devices=N_CORES)
    xu_d = nc.dram_tensor("xu_in", [9, NG * BPC], F32, kind="ExternalInput").ap()
    sel_d = nc.dram_tensor("sel_in", [9, 128], F32, kind="ExternalInput").ap()
    dc_d = nc.dram_tensor("dc_in", [128, NG * 64], F32, kind="ExternalInput").ap()
    out_d = nc.dram_tensor("out", [BPC, NG * 512], F32, kind="ExternalOutput").ap()

    with tile.TileContext(nc) as tc, ExitStack() as ctx:
        pool = ctx.enter_context(tc.tile_pool(name="main", bufs=1))
        psW = ctx.enter_context(tc.tile_pool(name="psW", bufs=1, space="PSUM"))
        psT = ctx.enter_context(tc.tile_pool(name="psT", bufs=2, space="PSUM"))
        psO = ctx.enter_context(tc.tile_pool(name="psO", bufs=4, space="PSUM"))
        og_pool = ctx.enter_context(tc.tile_pool(name="og", bufs=4))

        # input loads: xu + sel feed the broadcast matmul (critical path)
        xu_sb = pool.tile([9, NG * BPC], F32)
        nc.sync.dma_start(out=xu_sb[:], in_=xu_d)
        sel_sb = pool.tile([9, 128], F32)
        nc.scalar.dma_start(out=sel_sb[:], in_=sel_d)

        # block-diagonal rhs: zero it, then scatter the compact D bands in
        # straight from DRAM (per i_l: 16 partitions, cols g*512+i_l*64..+64)
        rhs_sb = pool.tile([128, NG * 512], F32)
        nc.vector.memset(rhs_sb[:, 0:2048], 0.0)
        nc.gpsimd.memset(rhs_sb[:, 2048:4096], 0.0)
        dcv = dc_d.rearrange("p (g o) -> p g o", o=64)
        rhv = rhs_sb[:].rearrange("p (g c) -> p g c", c=512)
        for il in range(8):
            eng = nc.sync if il % 2 == 0 else nc.scalar
            eng.dma_start(out=rhv[il * 16:(il + 1) * 16, :, il * 64:(il + 1) * 64],
                          in_=dcv[il * 16:(il + 1) * 16, :, :])

        # PE warmup: tiny matmuls on a zeroed tile keep the PE clocking from
        # t=0 so the real matmuls run at full p-state.
        warm = pool.tile([128, 16], F32)
        nc.scalar.memzero(warm[:])
        ps_w = psW.tile([16, 16], F32)
        for _ in range(NWARM):
            nc.tensor.matmul(out=ps_w[:], lhsT=warm[0:32, :], rhs=warm[0:32, :],
                             start=True, stop=True)

        basesT = pool.tile([128, NG * BPC], F32)
        for h in range(2):
            sl = slice(h * 512, (h + 1) * 512)
            ps_t = psT.tile([128, 512], F32)
            nc.tensor.matmul(out=ps_t[:], lhsT=sel_sb[:], rhs=xu_sb[:, sl],
                             start=True, stop=True)
            r_h = pool.tile([128, 512], F32)
            nc.scalar.activation(out=r_h[:], in_=ps_t[:],
                                 func=mybir.ActivationFunctionType.Relu)
            # t*relu(t) = relu(t)^2, and only one PSUM operand per op
            q_h = pool.tile([128, 512], F32)
            nc.vector.tensor_mul(q_h[:], ps_t[:], r_h[:])
            nc.vector.tensor_mul(basesT[:, sl], q_h[:], r_h[:])

        for g in range(NG):
            ps_o = psO.tile([128, 512], F32)
            nc.tensor.matmul(out=ps_o[:],
                             lhsT=basesT[:, g * BPC:(g + 1) * BPC],
                             rhs=rhs_sb[:, g * 512:(g + 1) * 512],
                             start=True, stop=True)
            og = og_pool.tile([128, 512], F32)
            if g % 2 == 0:
                nc.scalar.copy(og[:], ps_o[:])
            else:
                nc.vector.tensor_copy(og[:], ps_o[:])
            nc.sync.dma_start(out=out_d[:, g * 512:(g + 1) * 512], in_=og[:])

    nc.compile()
    return nc


def _host_inputs(x, coef, grid):
    x = np.asarray(x, dtype=np.float32)
    coef = np.asarray(coef, dtype=np.float32)
    knots = np.asarray(grid, dtype=np.float32)[0, 0, :]          # (23,)
    h = float(knots[1] - knots[0])

    u = ((x - knots[8]) / h).astype(np.float32)                  # (B, in)

    # D[o,i,n] = (1/6) sum_j w_j coef[o,i,8+n-j], n = 0..10 (rest zero)
    w = np.array([1.0, -4.0, 6.0, -4.0, 1.0], np.float32)
    C8 = coef[:, :, 8:19]                                        # (o,i,11)
    D16 = np.zeros((OUT_DIM, IN_DIM, K16), np.float32)
    for n in range(11):
        for j in range(5):
            m = n - j
            if 0 <= m <= 10:
                D16[:, :, n] += w[j] * C8[:, :, m]
    D16 /= 6.0

    # compact block bands: dc[i_l*16+j, g*64+o] = D16[o, g*8+i_l, j]
    dc = np.zeros((128, NG * 64), np.float32)
    for il in range(8):
        for g in range(NG):
            i = g * 8 + il
            dc[il * 16:il * 16 + K16, g * 64:(g + 1) * 64] = D16[:, i, :].T

    # sel[k,p] = (p//16 == k); sel[8,p] = -(p%16)
    p = np.arange(128)
    sel = np.zeros((9, 128), np.float32)
    sel[p // 16, p] = 1.0
    sel[8, :] = -(p % 16).astype(np.float32)
    return u, sel, dc


def _execute(x, coef, grid, trace=False, **spmd_kwargs):
    u, sel, dc = _host_inputs(x, coef, grid)
    if "nc" not in _CACHE:
        _CACHE["nc"] = _build_nc()
    nc = _CACHE["nc"]
    in_maps = []
    for c in range(N_CORES):
        uc = u[c * BPC:(c + 1) * BPC]                            # (128, 64)
        xu2 = np.empty((9, NG * BPC), np.float32)
        # row k<8: u[b, g*8+k] laid out col = g*128 + b
        xu2[:8] = uc.reshape(BPC, NG, 8).transpose(2, 1, 0).reshape(8, NG * BPC)
        xu2[8] = 1.0
        in_maps.append({"xu_in": np.ascontiguousarray(xu2),
                        "sel_in": sel, "dc_in": dc})
    res = run_bass_kernel_spmd(nc, in_maps, list(range(N_CORES)),
                               trace=trace, **spmd_kwargs)
    full = np.empty((B_TOT, OUT_DIM, IN_DIM), dtype=np.float32)
    for c in range(N_CORES):
        t = res.results[c]["out"].reshape(BPC, NG, 8, 64)        # (b, g, i_l, o)
        full[c * BPC:(c + 1) * BPC] = (
            t.transpose(0, 3, 1, 2).reshape(BPC, OUT_DIM, IN_DIM))
    return full, res


def kernel(x, coef, grid):
    out, _ = _execute(x, coef, grid, trace=False)
    return out
